# revision 22
# baseline (speedup 1.0000x reference)
"""Causal self-attention (B=2, T=2048, C=1024, H=16, D=64) on 8 trn2 cores.

Sharding: tensor-parallel over (batch, head-group). Core c handles batch
c//4 and heads 4*(c%4) .. 4*(c%4)+4. Each core computes its 4 heads'
QKV projection, causal attention, and the partial output projection
(W_proj row-shard). The 4 partials per batch are summed on the host
(Megatron all-reduce at gather time), where the constant bias terms
(b_proj and b_v @ W_proj) are also added.

On-core dataflow is feature-major, with X^T pre-transposed ON THE HOST
(uploaded t4-blocked: row t4*128+p, col c*512+tl), eliminating all PE
transposes. The Q/K projections run as fp8e4 DoubleRow matmuls (2 fp8
MACs/cell/cycle, contraction-pair interleave; x scaled by 8, wq/wk by
16 -> scores scaled 2^14, folded into the exp scale). Score noise
washes out in the softmax so fp8 is safe there; the V/PV/projection
path is noise-critical and runs fp16 from a separate fp16 X^T copy.

Attention per head pair (heads 2m, 2m+1 on partitions 0:64 / 64:128):
  S^T[k,q] = K Q^T per 128-k-chunk into 1024-wide PSUM groups, one ACT
  exp per (head, group), diag blocks masked on DVE. O'^T += V'^T P^T
  with V' = [V_h | ones] so rows 64:128 accumulate the softmax
  denominator. O^T = O'^T[0:64] * recip(denominator) per chunk tile;
  Y^T = W_proj^T O^T (fp16, partial; summed/biased on the host).
DMA triggers cost ~0.6us of HWDGE-sequencer time each and queue order
gates the early rounds, so round-0 dependencies are issued first and
split across the two HWDGE queues (sync: x-tiles; scalar: weights).
"""
import os
import sys
import numpy as np

B, T, C = 2, 2048, 1024
H, D = 16, 64
HPC = 4                 # heads per core
QC = HPC * D            # 256 qkv cols per core
NCORES = 8
NT = T // 128           # 16 T-chunks of 128
NT4 = T // 512          # 4 T-chunks of 512
NKC = 8                 # contraction chunks over C
SX = 8.0                # fp8 scale for x
SQK = 16.0              # fp8 scale for wq/wk
EXP_SCALE = 0.125 / (SX * SQK) ** 2     # 2^-17

_cache = {}


def _ensure_env():
    for p in ("/opt/trn_rl_repo", "/root/.axon_site/_ro/trn_rl_repo"):
        if os.path.isdir(p) and p not in sys.path:
            sys.path.append(p)
    jp = os.environ.get("JAX_PLATFORMS")
    if jp and "axon" not in jp and "jax" not in sys.modules:
        os.environ["JAX_PLATFORMS"] = ""


def _make_groups(t4):
    """Pack S chunks for q-chunk t4 into 1024-wide PSUM half-tiles so
    no chunk straddles a 512 (bank) boundary."""
    lo0, hi0 = t4 * 512, (t4 + 1) * 512
    last_kc = 4 * t4 + 3
    groups, cur, pos = [], [], 0
    for kc in range(last_kc + 1):
        lo = max(lo0, kc * 128)
        n = hi0 - lo
        npos = pos if pos % 512 + n <= 512 else (pos + 511) // 512 * 512
        if npos + n > 1024:
            groups.append(cur)
            cur, npos = [], 0
        cur.append((kc, lo, n, npos))
        pos = npos + n
    groups.append(cur)
    return groups


def _build():
    import concourse.bass as bass
    import concourse.bacc as bacc
    import concourse.mybir as mybir
    import concourse.tile as tile

    F32 = mybir.dt.float32
    F16 = mybir.dt.float16
    F8 = mybir.dt.float8e4
    AF = mybir.ActivationFunctionType
    DR = mybir.MatmulPerfMode.DoubleRow

    nc = bacc.Bacc()
    x_d = nc.dram_tensor("x", [4 * 128, NKC * 512], F8, kind="ExternalInput")
    x16_d = nc.dram_tensor("x16", [4 * 128, NKC * 512], F16,
                           kind="ExternalInput")
    wq_d = nc.dram_tensor("wq", [128, NKC * QC], F8, kind="ExternalInput")
    wk_d = nc.dram_tensor("wk", [128, NKC * QC], F8, kind="ExternalInput")
    wv_d = nc.dram_tensor("wv", [128, NKC * QC], F16, kind="ExternalInput")
    wp_d = nc.dram_tensor("wp", [128, 2 * C], F16, kind="ExternalInput")
    bq_d = nc.dram_tensor("bq", [128, 2], F32, kind="ExternalInput")
    bk_d = nc.dram_tensor("bk", [128, 2], F32, kind="ExternalInput")
    mask_d = nc.dram_tensor("mask", [128, 128], F16, kind="ExternalInput")
    on_d = nc.dram_tensor("ones16", [128, NT * QC], F16,
                          kind="ExternalInput")
    yt_d = nc.dram_tensor("yt", [C, T], F16, kind="ExternalOutput")

    with tile.TileContext(nc) as tc:
        with tc.tile_pool(name="cst", bufs=1) as cst, \
             tc.tile_pool(name="wgt", bufs=1) as wgt, \
             tc.tile_pool(name="xin", bufs=1) as xin, \
             tc.tile_pool(name="qk", bufs=1) as qkp, \
             tc.tile_pool(name="vv", bufs=1) as vvp, \
             tc.tile_pool(name="pp", bufs=6) as ppp, \
             tc.tile_pool(name="dn", bufs=4) as dnp, \
             tc.tile_pool(name="yy", bufs=4) as yyp, \
             tc.tile_pool(name="mm", bufs=2, space="PSUM") as mmp, \
             tc.tile_pool(name="ss", bufs=2, space="PSUM") as ssp, \
             tc.tile_pool(name="po", bufs=2, space="PSUM") as pop:

            # ---- inputs; issue order tuned so round-0 deps land first:
            # scalar queue: wq, x16_0, wk, bq, bk, wv, mask, x16_1,
            # ones, x16_2, x16_3, wp.  sync queue: xt0..3 (fp8). ----
            xt_s = [xin.tile([128, NKC * 512], F8, tag=f"xt{t4}",
                             name=f"xt{t4}") for t4 in range(NT4)]
            x16_s = [xin.tile([128, NKC * 512], F16, tag=f"xs{t4}",
                              name=f"xs{t4}") for t4 in range(NT4)]

            def ld16(t4):
                nc.sync.dma_start(out=x16_s[t4][:],
                                  in_=x16_d[t4 * 128:(t4 + 1) * 128, :])

            # sync queue: wq, wk, bq, bk, xt0, xt1, x16_0, xt2, xt3,
            # x16_1..3 -- every round-0-critical tensor at minimal
            # queue position on the early-draining queue
            wq_s = wgt.tile([128, NKC * QC], F8, tag="wq")
            nc.sync.dma_start(out=wq_s[:], in_=wq_d[:])
            nc.sync.dma_start(out=xt_s[0][:], in_=x_d[0:128, :])
            wk_s = wgt.tile([128, NKC * QC], F8, tag="wk")
            nc.sync.dma_start(out=wk_s[:], in_=wk_d[:])
            bq_s = cst.tile([128, 2], F32, tag="bq")
            nc.sync.dma_start(out=bq_s[:], in_=bq_d[:])
            bk_s = cst.tile([128, 2], F32, tag="bk")
            nc.sync.dma_start(out=bk_s[:], in_=bk_d[:])
            nc.sync.dma_start(out=xt_s[1][:], in_=x_d[128:256, :])
            ld16(0)
            nc.sync.dma_start(out=xt_s[2][:], in_=x_d[256:384, :])
            nc.sync.dma_start(out=xt_s[3][:], in_=x_d[384:512, :])
            ld16(1)
            ld16(2)
            ld16(3)
            xt4 = [xt_s[t4][:].rearrange("p (c t) -> p c t", t=512)
                   for t4 in range(NT4)]
            xs4 = [x16_s[t4][:].rearrange("p (c t) -> p c t", t=512)
                   for t4 in range(NT4)]

            # scalar queue: wv, mask (late-needed only)
            wv_s = wgt.tile([128, NKC * QC], F16, tag="wv")
            nc.scalar.dma_start(out=wv_s[:], in_=wv_d[:])
            mask = cst.tile([128, 128], F16, tag="mask")
            nc.scalar.dma_start(out=mask[:], in_=mask_d[:])

            wp_s = wgt.tile([128, 2 * C], F16, tag="wp")

            wq3 = wq_s[:].rearrange("p (c n) -> p c n", n=QC)
            wk3 = wk_s[:].rearrange("p (c n) -> p c n", n=QC)
            wv3 = wv_s[:].rearrange("p (c n) -> p c n", n=QC)
            wp3 = wp_s[:].rearrange("p (k n) -> p k n", n=C)

            # ---- persistent activations ----
            qt_s = [qkp.tile([128, T], F16, tag=f"qt{m}", name=f"qt{m}")
                    for m in range(2)]
            kt_s = [qkp.tile([128, T], F16, tag=f"kt{m}", name=f"kt{m}")
                    for m in range(2)]
            # O^T as separate tiles per (head-pair, t4 chunk) so the
            # output projection of chunk n4 only depends on round n4
            ot_s = [[qkp.tile([128, 512], F16, tag=f"ot{k}_{t}",
                              name=f"ot{k}_{t}") for t in range(NT4)]
                    for k in range(2)]
            # V' tiles: per head 64 V cols + 64 ones cols -> [128, 512]
            vp_s = [vvp.tile([128, HPC * 2 * D], F16, tag=f"vp{i}",
                             name=f"vp{i}") for i in range(NT)]
            for i in range(NT):
                v3i = vp_s[i][:].rearrange("p (h e) -> p h e", e=2 * D)
                nc.sync.dma_start(
                    out=v3i[:, :, D:2 * D],
                    in_=on_d.ap()[:, 0:QC].rearrange("p (h d) -> p h d", d=D))
            nc.scalar.dma_start(out=wp_s[:], in_=wp_d[:])

            def proj_emit(n4):
                for mo in range(8):
                    py = mmp.tile([128, 512], F32, tag="mm", name="py")
                    for k in range(2):
                        nc.tensor.matmul(py[:],
                                         wp3[:, k, mo * 128:(mo + 1) * 128],
                                         ot_s[k][n4][:],
                                         start=(k == 0), stop=(k == 1))
                    ys = yyp.tile([128, 512], F16, tag="yt", name="ys")
                    if mo % 2 == 0:
                        nc.vector.tensor_copy(ys[:], py[:])
                    else:
                        nc.scalar.copy(ys[:], py[:])
                    nc.sync.dma_start(
                        out=yt_d[mo * 128:(mo + 1) * 128,
                                 n4 * 512:(n4 + 1) * 512],
                        in_=ys[:])

            # ---- fused rounds over 512-wide T-chunks ----
            for t4 in range(NT4):
                lo0, hi0 = t4 * 512, (t4 + 1) * 512
                # Q^T, K^T chunks [128, 512] (fp8 DoubleRow over c-pairs)
                for (w3, bs, dst) in ((wq3, bq_s, qt_s), (wk3, bk_s, kt_s)):
                    for m in range(2):
                        pq = mmp.tile([128, 512], F32, tag="mm", name="pq")
                        for j in range(NKC // 2):
                            nc.tensor.matmul(
                                pq[:],
                                w3[:, 2 * j:2 * j + 2, m * 128:(m + 1) * 128],
                                xt4[t4][:, 2 * j:2 * j + 2, :],
                                start=(j == 0), stop=(j == NKC // 2 - 1),
                                perf_mode=DR)
                        nc.vector.tensor_scalar_add(
                            dst[m][:, lo0:hi0], pq[:], bs[:, m:m + 1])
                # V natural [128, 256] per 128-T-subchunk (fp16 path)
                for i in range(4):
                    kc = 4 * t4 + i
                    pv = mmp.tile([128, 512], F32, tag="mm", name="pv")
                    for c in range(NKC):
                        nc.tensor.matmul(
                            pv[:, 0:QC],
                            xs4[t4][:, c, i * 128:(i + 1) * 128],
                            wv3[:, c],
                            start=(c == 0), stop=(c == NKC - 1))
                    v3 = vp_s[kc][:].rearrange("p (h e) -> p h e", e=2 * D)
                    nc.vector.tensor_copy(
                        v3[:, :, 0:D],
                        pv[:, 0:QC].rearrange("p (h d) -> p h d", d=D))

                # attention for q-chunk t4, head pairs (S^T layout; O'
                # rows 0:64 = V-accum, rows 64:128 = denominator).
                groups = _make_groups(t4)
                last_kc = 4 * t4 + 3
                for m in range(2):
                    op_tl = [pop.tile([128, 512], F32, tag="po",
                                      name=f"op{t4}_{m}_{e}")
                             for e in range(2)]
                    for grp in groups:
                        sp = [ssp.tile([128, 1024], F32, tag="ss",
                                       name=f"sp{e}") for e in range(2)]
                        for (kc, lo, n, off) in grp:
                            for e in range(2):
                                r0, r1 = e * 64, e * 64 + 64
                                nc.tensor.matmul(
                                    sp[e][:, off:off + n],
                                    kt_s[m][r0:r1, kc * 128:kc * 128 + 128],
                                    qt_s[m][r0:r1, lo:hi0],
                                    start=True, stop=True)
                        end = grp[-1][3] + grp[-1][2]
                        pt = [ppp.tile([128, 1024], F16, tag="p",
                                       name=f"pt{e}") for e in range(2)]
                        for e in range(2):
                            nc.scalar.activation(pt[e][:, 0:end],
                                                 sp[e][:, 0:end],
                                                 AF.Exp, scale=float(EXP_SCALE))
                        for (kc, lo, n, off) in grp:
                            if kc * 128 >= lo0:  # diagonal block
                                for e in range(2):
                                    nc.vector.tensor_mul(
                                        pt[e][:, off:off + 128],
                                        pt[e][:, off:off + 128], mask[:])
                            for e in range(2):
                                h = 2 * m + e
                                nc.tensor.matmul(
                                    op_tl[e][:, lo - lo0:512],
                                    vp_s[kc][:, h * 2 * D:(h + 1) * 2 * D],
                                    pt[e][:, off:off + n],
                                    start=(kc == 0), stop=(kc == last_kc))
                    # normalize chunk t4 of the head pair
                    for e in range(2):
                        rci = dnp.tile([64, 512], F32, tag="rci",
                                       name=f"rci{e}")
                        if t4 == NT4 - 1 and m == 1:
                            nc.scalar.copy(rci[:], op_tl[e][D:2 * D, :])
                        else:
                            nc.vector.tensor_copy(rci[:], op_tl[e][D:2 * D, :])
                        rc = dnp.tile([64, 512], F32, tag="rc", name=f"rc{e}")
                        nc.vector.reciprocal_approx_fast(rc[:], rci[:])
                        nc.vector.tensor_mul(
                            ot_s[m][t4][e * 64:e * 64 + 64, :],
                            op_tl[e][0:D, :], rc[:])

                proj_emit(t4)

    nc.finalize()
    return nc


def _get_program():
    if "nc" not in _cache:
        _ensure_env()
        _cache["nc"] = _build()
    return _cache["nc"]


def kernel(x, w_attn, b_attn, w_proj, b_proj):
    import ml_dtypes
    F8NP = ml_dtypes.float8_e4m3

    x = np.asarray(x, dtype=np.float32)
    w_attn = np.asarray(w_attn, dtype=np.float32)
    b_attn = np.asarray(b_attn, dtype=np.float32)
    w_proj = np.asarray(w_proj, dtype=np.float32)
    b_proj = np.asarray(b_proj, dtype=np.float32)

    nc = _get_program()
    from concourse.bass_utils import run_bass_kernel_spmd

    mask = np.triu(np.ones((128, 128), dtype=np.float16))

    # x^T in t4-blocked layout per batch: [t4*128+p, c*512+tl]
    xt_host, x16_host = [], []
    for b in range(B):
        a = x[b].reshape(NT4, 512, NKC, 128).transpose(0, 3, 2, 1)
        a = a.reshape(NT4 * 128, NKC * 512)
        xt_host.append(np.ascontiguousarray((a * SX).astype(F8NP)))
        x16_host.append(np.ascontiguousarray(a.astype(np.float16)))

    def wsplit(w, s, dt):  # [C, QC] -> [128, NKC*QC], (c p) n -> p (c n)
        return np.ascontiguousarray(
            (w * s).astype(dt).reshape(NKC, 128, QC)
            .transpose(1, 0, 2).reshape(128, NKC * QC))

    in_maps = []
    for c in range(NCORES):
        b = c // 4
        hg = c % 4
        q0 = hg * QC
        wp_c = w_proj[q0:q0 + QC, :].astype(np.float16)  # [256, 1024]
        in_maps.append({
            "x": xt_host[b],
            "x16": x16_host[b],
            "wq": wsplit(w_attn[:, q0:q0 + QC], SQK, F8NP),
            "wk": wsplit(w_attn[:, C + q0:C + q0 + QC], SQK, F8NP),
            "wv": wsplit(w_attn[:, 2 * C + q0:2 * C + q0 + QC], 1.0,
                         np.float16),
            "wp": np.ascontiguousarray(
                wp_c.reshape(2, 128, C).transpose(1, 0, 2).reshape(128, 2 * C)),
            "bq": np.ascontiguousarray(
                (b_attn[q0:q0 + QC] * SX * SQK).reshape(2, 128).T
                .astype(np.float32)),
            "bk": np.ascontiguousarray(
                (b_attn[C + q0:C + q0 + QC] * SX * SQK).reshape(2, 128).T
                .astype(np.float32)),
            "mask": mask,
            "ones16": np.ones((128, NT * QC), dtype=np.float16),
        })

    trace = bool(os.environ.get("KERNEL_TRACE"))
    res = run_bass_kernel_spmd(nc, in_maps, list(range(NCORES)), trace=trace)
    _cache["last_results"] = res

    # constant bias: b_proj + b_v @ w_proj (exact: the ones-column
    # denominator normalization makes the V-bias contribution 1*b_v)
    bvp = (b_attn[2 * C:] @ w_proj + b_proj).astype(np.float32)

    out = np.empty((B, T, C), dtype=np.float32)
    for b in range(B):
        acc = res.results[4 * b]["yt"].astype(np.float32)
        for c in range(4 * b + 1, 4 * b + 4):
            acc = acc + res.results[c]["yt"].astype(np.float32)
        out[b] = acc.T + bvp
    return out


# revision 23
# speedup vs baseline: 1.2904x; 1.2904x over previous
"""Causal self-attention (B=2, T=2048, C=1024, H=16, D=64) on 8 trn2 cores.

Sharding: tensor-parallel over (batch, head-group). Core c handles batch
c//4 and heads 4*(c%4) .. 4*(c%4)+4. Each core computes its 4 heads'
QKV projection, causal attention, and the partial output projection
(W_proj row-shard). The 4 partials per batch are summed on the host
(Megatron all-reduce at gather time), where the constant bias terms
(b_proj and b_v @ W_proj) are also added.

On-core dataflow is feature-major, with X^T pre-transposed ON THE HOST
(uploaded t4-blocked: row t4*128+p, col c*512+tl), eliminating all PE
transposes. The Q/K projections run as fp8e4 DoubleRow matmuls (2 fp8
MACs/cell/cycle, contraction-pair interleave; x scaled by 8, wq/wk by
16 -> scores scaled 2^14, folded into the exp scale). Score noise
washes out in the softmax so fp8 is safe there; the V/PV/projection
path is noise-critical and runs fp16 from a separate fp16 X^T copy.

Attention per head pair (heads 2m, 2m+1 on partitions 0:64 / 64:128):
  S^T[k,q] = K Q^T per 128-k-chunk into 1024-wide PSUM groups, one ACT
  exp per (head, group), diag blocks masked on DVE. O'^T += V'^T P^T
  with V' = [V_h | ones] so rows 64:128 accumulate the softmax
  denominator. O^T = O'^T[0:64] * recip(denominator) per chunk tile;
  Y^T = W_proj^T O^T (fp16, partial; summed/biased on the host).
DMA triggers cost ~0.6us of HWDGE-sequencer time each and queue order
gates the early rounds, so round-0 dependencies are issued first and
split across the two HWDGE queues (sync: x-tiles; scalar: weights).
"""
import os
import sys
import numpy as np

B, T, C = 2, 2048, 1024
H, D = 16, 64
HPC = 4                 # heads per core
QC = HPC * D            # 256 qkv cols per core
NCORES = 8
NT = T // 128           # 16 T-chunks of 128
NT4 = T // 512          # 4 T-chunks of 512
NKC = 8                 # contraction chunks over C
SX = 8.0                # fp8 scale for x
SQK = 16.0              # fp8 scale for wq/wk
EXP_SCALE = 0.125 / (SX * SQK) ** 2     # 2^-17

_cache = {}


def _ensure_env():
    for p in ("/opt/trn_rl_repo", "/root/.axon_site/_ro/trn_rl_repo"):
        if os.path.isdir(p) and p not in sys.path:
            sys.path.append(p)
    jp = os.environ.get("JAX_PLATFORMS")
    if jp and "axon" not in jp and "jax" not in sys.modules:
        os.environ["JAX_PLATFORMS"] = ""


def _make_groups(t4):
    """Pack S chunks for q-chunk t4 into 1024-wide PSUM half-tiles so
    no chunk straddles a 512 (bank) boundary."""
    lo0, hi0 = t4 * 512, (t4 + 1) * 512
    last_kc = 4 * t4 + 3
    groups, cur, pos = [], [], 0
    for kc in range(last_kc + 1):
        lo = max(lo0, kc * 128)
        n = hi0 - lo
        npos = pos if pos % 512 + n <= 512 else (pos + 511) // 512 * 512
        if npos + n > 1024:
            groups.append(cur)
            cur, npos = [], 0
        cur.append((kc, lo, n, npos))
        pos = npos + n
    groups.append(cur)
    return groups


def _build():
    import concourse.bass as bass
    import concourse.bacc as bacc
    import concourse.mybir as mybir
    import concourse.tile as tile

    F32 = mybir.dt.float32
    F16 = mybir.dt.float16
    F8 = mybir.dt.float8e4
    AF = mybir.ActivationFunctionType
    DR = mybir.MatmulPerfMode.DoubleRow

    nc = bacc.Bacc()
    x_d = nc.dram_tensor("x", [4 * 128, NKC * 512], F8, kind="ExternalInput")
    x16_d = nc.dram_tensor("x16", [4 * 128, NKC * 512], F16,
                           kind="ExternalInput")
    wq_d = nc.dram_tensor("wq", [128, NKC * QC], F8, kind="ExternalInput")
    wk_d = nc.dram_tensor("wk", [128, NKC * QC], F8, kind="ExternalInput")
    wv_d = nc.dram_tensor("wv", [128, NKC * QC], F16, kind="ExternalInput")
    wp_d = nc.dram_tensor("wp", [128, 2 * C], F16, kind="ExternalInput")
    bq_d = nc.dram_tensor("bq", [128, 2], F32, kind="ExternalInput")
    bk_d = nc.dram_tensor("bk", [128, 2], F32, kind="ExternalInput")
    mask_d = nc.dram_tensor("mask", [128, 128], F16, kind="ExternalInput")
    on_d = nc.dram_tensor("ones16", [128, NT * QC], F16,
                          kind="ExternalInput")
    yt_d = nc.dram_tensor("yt", [C, T], F16, kind="ExternalOutput")

    with tile.TileContext(nc) as tc:
        with tc.tile_pool(name="cst", bufs=1) as cst, \
             tc.tile_pool(name="wgt", bufs=1) as wgt, \
             tc.tile_pool(name="xin", bufs=1) as xin, \
             tc.tile_pool(name="qk", bufs=1) as qkp, \
             tc.tile_pool(name="vv", bufs=1) as vvp, \
             tc.tile_pool(name="pp", bufs=6) as ppp, \
             tc.tile_pool(name="dn", bufs=4) as dnp, \
             tc.tile_pool(name="yy", bufs=4) as yyp, \
             tc.tile_pool(name="mm", bufs=2, space="PSUM") as mmp, \
             tc.tile_pool(name="ss", bufs=2, space="PSUM") as ssp, \
             tc.tile_pool(name="po", bufs=2, space="PSUM") as pop:

            # ---- inputs; issue order tuned so round-0 deps land first:
            # scalar queue: wq, x16_0, wk, bq, bk, wv, mask, x16_1,
            # ones, x16_2, x16_3, wp.  sync queue: xt0..3 (fp8). ----
            xt_s = [xin.tile([128, NKC * 512], F8, tag=f"xt{t4}",
                             name=f"xt{t4}") for t4 in range(NT4)]
            x16_s = [xin.tile([128, NKC * 512], F16, tag=f"xs{t4}",
                              name=f"xs{t4}") for t4 in range(NT4)]

            def ld16(t4):
                nc.sync.dma_start(out=x16_s[t4][:],
                                  in_=x16_d[t4 * 128:(t4 + 1) * 128, :])

            # sync queue: wq, wk, bq, bk, xt0, xt1, x16_0, xt2, xt3,
            # x16_1..3 -- every round-0-critical tensor at minimal
            # queue position on the early-draining queue
            wq_s = wgt.tile([128, NKC * QC], F8, tag="wq")
            nc.sync.dma_start(out=wq_s[:], in_=wq_d[:])
            nc.sync.dma_start(out=xt_s[0][:], in_=x_d[0:128, :])
            wk_s = wgt.tile([128, NKC * QC], F8, tag="wk")
            nc.sync.dma_start(out=wk_s[:], in_=wk_d[:])
            bq_s = cst.tile([128, 2], F32, tag="bq")
            nc.sync.dma_start(out=bq_s[:], in_=bq_d[:])
            bk_s = cst.tile([128, 2], F32, tag="bk")
            nc.sync.dma_start(out=bk_s[:], in_=bk_d[:])
            nc.sync.dma_start(out=xt_s[1][:], in_=x_d[128:256, :])
            ld16(0)
            nc.sync.dma_start(out=xt_s[2][:], in_=x_d[256:384, :])
            nc.sync.dma_start(out=xt_s[3][:], in_=x_d[384:512, :])
            ld16(1)
            ld16(2)
            ld16(3)
            xt4 = [xt_s[t4][:].rearrange("p (c t) -> p c t", t=512)
                   for t4 in range(NT4)]
            xs4 = [x16_s[t4][:].rearrange("p (c t) -> p c t", t=512)
                   for t4 in range(NT4)]

            # scalar queue: wv, mask (late-needed only)
            wv_s = wgt.tile([128, NKC * QC], F16, tag="wv")
            nc.scalar.dma_start(out=wv_s[:], in_=wv_d[:])
            mask = cst.tile([128, 128], F16, tag="mask")
            nc.scalar.dma_start(out=mask[:], in_=mask_d[:])

            wp_s = wgt.tile([128, 2 * C], F16, tag="wp")

            wq3 = wq_s[:].rearrange("p (c n) -> p c n", n=QC)
            wk3 = wk_s[:].rearrange("p (c n) -> p c n", n=QC)
            wv3 = wv_s[:].rearrange("p (c n) -> p c n", n=QC)
            wp3 = wp_s[:].rearrange("p (k n) -> p k n", n=C)

            # ---- persistent activations ----
            qt_s = [qkp.tile([128, T], F16, tag=f"qt{m}", name=f"qt{m}")
                    for m in range(2)]
            kt_s = [qkp.tile([128, T], F16, tag=f"kt{m}", name=f"kt{m}")
                    for m in range(2)]
            # O^T as separate tiles per (head-pair, t4 chunk) so the
            # output projection of chunk n4 only depends on round n4
            ot_s = [[qkp.tile([128, 512], F16, tag=f"ot{k}_{t}",
                              name=f"ot{k}_{t}") for t in range(NT4)]
                    for k in range(2)]
            # V' tiles: per head 64 V cols + 64 ones cols -> [128, 512]
            vp_s = [vvp.tile([128, HPC * 2 * D], F16, tag=f"vp{i}",
                             name=f"vp{i}") for i in range(NT)]
            for i in range(NT):
                v3i = vp_s[i][:].rearrange("p (h e) -> p h e", e=2 * D)
                nc.sync.dma_start(
                    out=v3i[:, :, D:2 * D],
                    in_=on_d.ap()[:, 0:QC].rearrange("p (h d) -> p h d", d=D))
            nc.scalar.dma_start(out=wp_s[:], in_=wp_d[:])

            # ---- fused rounds over 512-wide T-chunks ----
            for t4 in range(NT4):
                lo0, hi0 = t4 * 512, (t4 + 1) * 512
                # Q^T, K^T chunks [128, 512] (fp8 DoubleRow over c-pairs)
                for (w3, bs, dst) in ((wq3, bq_s, qt_s), (wk3, bk_s, kt_s)):
                    for m in range(2):
                        pq = mmp.tile([128, 512], F32, tag="mm", name="pq")
                        for j in range(NKC // 2):
                            nc.tensor.matmul(
                                pq[:],
                                w3[:, 2 * j:2 * j + 2, m * 128:(m + 1) * 128],
                                xt4[t4][:, 2 * j:2 * j + 2, :],
                                start=(j == 0), stop=(j == NKC // 2 - 1),
                                perf_mode=DR)
                        nc.vector.tensor_scalar_add(
                            dst[m][:, lo0:hi0], pq[:], bs[:, m:m + 1])
                # V natural [128, 256] per 128-T-subchunk (fp16 path)
                for i in range(4):
                    kc = 4 * t4 + i
                    pv = mmp.tile([128, 512], F32, tag="mm", name="pv")
                    for c in range(NKC):
                        nc.tensor.matmul(
                            pv[:, 0:QC],
                            xs4[t4][:, c, i * 128:(i + 1) * 128],
                            wv3[:, c],
                            start=(c == 0), stop=(c == NKC - 1))
                    v3 = vp_s[kc][:].rearrange("p (h e) -> p h e", e=2 * D)
                    nc.vector.tensor_copy(
                        v3[:, :, 0:D],
                        pv[:, 0:QC].rearrange("p (h d) -> p h d", d=D))

                # attention for q-chunk t4, head pairs (S^T layout; O'
                # rows 0:64 = V-accum, rows 64:128 = denominator).
                groups = _make_groups(t4)
                last_kc = 4 * t4 + 3
                for m in range(2):
                    op_tl = [pop.tile([128, 512], F32, tag="po",
                                      name=f"op{t4}_{m}_{e}")
                             for e in range(2)]
                    for grp in groups:
                        sp = [ssp.tile([128, 1024], F32, tag="ss",
                                       name=f"sp{e}") for e in range(2)]
                        for (kc, lo, n, off) in grp:
                            for e in range(2):
                                r0, r1 = e * 64, e * 64 + 64
                                nc.tensor.matmul(
                                    sp[e][:, off:off + n],
                                    kt_s[m][r0:r1, kc * 128:kc * 128 + 128],
                                    qt_s[m][r0:r1, lo:hi0],
                                    start=True, stop=True)
                        end = grp[-1][3] + grp[-1][2]
                        pt = [ppp.tile([128, 1024], F16, tag="p",
                                       name=f"pt{e}") for e in range(2)]
                        for e in range(2):
                            nc.scalar.activation(pt[e][:, 0:end],
                                                 sp[e][:, 0:end],
                                                 AF.Exp, scale=float(EXP_SCALE))
                        for (kc, lo, n, off) in grp:
                            if kc * 128 >= lo0:  # diagonal block
                                for e in range(2):
                                    nc.vector.tensor_mul(
                                        pt[e][:, off:off + 128],
                                        pt[e][:, off:off + 128], mask[:])
                            for e in range(2):
                                h = 2 * m + e
                                nc.tensor.matmul(
                                    op_tl[e][:, lo - lo0:512],
                                    vp_s[kc][:, h * 2 * D:(h + 1) * 2 * D],
                                    pt[e][:, off:off + n],
                                    start=(kc == 0), stop=(kc == last_kc))
                    # normalize chunk t4 of the head pair
                    for e in range(2):
                        rci = dnp.tile([64, 512], F32, tag="rci",
                                       name=f"rci{e}")
                        if t4 == NT4 - 1 and m == 1:
                            nc.scalar.copy(rci[:], op_tl[e][D:2 * D, :])
                        else:
                            nc.vector.tensor_copy(rci[:], op_tl[e][D:2 * D, :])
                        rc = dnp.tile([64, 512], F32, tag="rc", name=f"rc{e}")
                        nc.vector.reciprocal_approx_fast(rc[:], rci[:])
                        nc.vector.tensor_mul(
                            ot_s[m][t4][e * 64:e * 64 + 64, :],
                            op_tl[e][0:D, :], rc[:])

            # ---- output projection (fp16, emitted last to fill the PE
            # during the exp-bound tail of the final attention round) ----
            for n4 in range(NT4):
                lo0, hi0 = n4 * 512, (n4 + 1) * 512
                for mo in range(8):
                    py = mmp.tile([128, 512], F32, tag="mm", name="py")
                    for k in range(2):
                        nc.tensor.matmul(py[:],
                                         wp3[:, k, mo * 128:(mo + 1) * 128],
                                         ot_s[k][n4][:],
                                         start=(k == 0), stop=(k == 1))
                    ys = yyp.tile([128, 512], F16, tag="yt", name="ys")
                    if mo % 2 == 0:
                        nc.vector.tensor_copy(ys[:], py[:])
                    else:
                        nc.scalar.copy(ys[:], py[:])
                    nc.sync.dma_start(
                        out=yt_d[mo * 128:(mo + 1) * 128, lo0:hi0],
                        in_=ys[:])

    nc.finalize()
    return nc


def _get_program():
    if "nc" not in _cache:
        _ensure_env()
        _cache["nc"] = _build()
    return _cache["nc"]


def kernel(x, w_attn, b_attn, w_proj, b_proj):
    import ml_dtypes
    F8NP = ml_dtypes.float8_e4m3

    x = np.asarray(x, dtype=np.float32)
    w_attn = np.asarray(w_attn, dtype=np.float32)
    b_attn = np.asarray(b_attn, dtype=np.float32)
    w_proj = np.asarray(w_proj, dtype=np.float32)
    b_proj = np.asarray(b_proj, dtype=np.float32)

    nc = _get_program()
    from concourse.bass_utils import run_bass_kernel_spmd

    mask = np.triu(np.ones((128, 128), dtype=np.float16))

    # x^T in t4-blocked layout per batch: [t4*128+p, c*512+tl]
    xt_host, x16_host = [], []
    for b in range(B):
        a = x[b].reshape(NT4, 512, NKC, 128).transpose(0, 3, 2, 1)
        a = a.reshape(NT4 * 128, NKC * 512)
        xt_host.append(np.ascontiguousarray((a * SX).astype(F8NP)))
        x16_host.append(np.ascontiguousarray(a.astype(np.float16)))

    def wsplit(w, s, dt):  # [C, QC] -> [128, NKC*QC], (c p) n -> p (c n)
        return np.ascontiguousarray(
            (w * s).astype(dt).reshape(NKC, 128, QC)
            .transpose(1, 0, 2).reshape(128, NKC * QC))

    in_maps = []
    for c in range(NCORES):
        b = c // 4
        hg = c % 4
        q0 = hg * QC
        wp_c = w_proj[q0:q0 + QC, :].astype(np.float16)  # [256, 1024]
        in_maps.append({
            "x": xt_host[b],
            "x16": x16_host[b],
            "wq": wsplit(w_attn[:, q0:q0 + QC], SQK, F8NP),
            "wk": wsplit(w_attn[:, C + q0:C + q0 + QC], SQK, F8NP),
            "wv": wsplit(w_attn[:, 2 * C + q0:2 * C + q0 + QC], 1.0,
                         np.float16),
            "wp": np.ascontiguousarray(
                wp_c.reshape(2, 128, C).transpose(1, 0, 2).reshape(128, 2 * C)),
            "bq": np.ascontiguousarray(
                (b_attn[q0:q0 + QC] * SX * SQK).reshape(2, 128).T
                .astype(np.float32)),
            "bk": np.ascontiguousarray(
                (b_attn[C + q0:C + q0 + QC] * SX * SQK).reshape(2, 128).T
                .astype(np.float32)),
            "mask": mask,
            "ones16": np.ones((128, NT * QC), dtype=np.float16),
        })

    trace = bool(os.environ.get("KERNEL_TRACE"))
    res = run_bass_kernel_spmd(nc, in_maps, list(range(NCORES)), trace=trace)
    _cache["last_results"] = res

    # constant bias: b_proj + b_v @ w_proj (exact: the ones-column
    # denominator normalization makes the V-bias contribution 1*b_v)
    bvp = (b_attn[2 * C:] @ w_proj + b_proj).astype(np.float32)

    out = np.empty((B, T, C), dtype=np.float32)
    for b in range(B):
        acc = res.results[4 * b]["yt"].astype(np.float32)
        for c in range(4 * b + 1, 4 * b + 4):
            acc = acc + res.results[c]["yt"].astype(np.float32)
        out[b] = acc.T + bvp
    return out
